# revision 9
# baseline (speedup 1.0000x reference)
"""Trainium2 Bass kernel for nn_Attention_59708635349115.

Decoder self-attention (GQA 16 q-heads / 4 kv-heads, RoPE, causal) over
B=2, S=2048, H=2048, distributed over 8 NeuronCores as 2 (batch) x 4
(head-group) shards.  Each core computes q/k/v projections for its
4 q-heads / 1 kv head, causal attention, and a partial o-projection
against its 512-row slice of Wo; the host sums the 4 partials per batch
(no on-device collectives).

v2 design vs the fp32r baseline:
  - bf16 operands everywhere (PSUM accumulation stays fp32); the host
    pre-packs every input into its exact SBUF layout so each tensor is
    one (or few) large contiguous DMA.
  - V is projected directly into [token, head_dim] layout by making the
    x chunk the stationary matmul operand - no PE transposes at all.
  - softmax normalization: 1/r is broadcast across partitions with a
    1-partition PE matmul (ones_row x recip_row) instead of the slow
    gpsimd partition_broadcast, then folded into the PSUM->SBUF copy of
    the PV output on DVE.
  - elementwise work split across DVE (RoPE muls, tri mask, normalize)
    and Pool/gpsimd (RoPE sin-mul, o_proj PSUM drain) so ACT only runs
    exp + a few copies (all in the same activation table - no reloads).
  - phases are software-pipelined per 512-token chunk jq:
    A(jq) projections -> C(jq-1) o-proj -> B(jq) attention, keeping PE
    fed across cross-engine dependency chains.
"""

import os
import sys

for _p in ("/opt/trn_rl_repo", "/root/.axon_site/_ro/trn_rl_repo"):
    if os.path.isdir(_p) and _p not in sys.path:
        sys.path.insert(0, _p)

import numpy as np
import ml_dtypes

import concourse.bass as bass
import concourse.mybir as mybir
import concourse.tile as tile
from concourse import bacc
from concourse.bass_utils import run_bass_kernel_spmd

B, S, H = 2, 2048, 2048
NH, NKV = 16, 4
HD = H // NH            # 128
G = 4                   # head-group shards (tensor parallel)
HPC = NH // G           # 4 q heads per core
N_CORES = 8
P = 128                 # partition dim
NQ = 512                # q-chunk (matmul moving dim)
NJ = S // NQ            # 4 q-chunks
KC = S // P             # 16 key/token 128-chunks
HC = H // P             # 16 hidden 128-chunks

F32 = mybir.dt.float32
F32R = mybir.dt.float32r
BF16 = mybir.dt.bfloat16
AF = mybir.ActivationFunctionType
BD = ml_dtypes.bfloat16

_CACHE = {}


def _build_program(loop_n=1, hw_loop=True):
    nc = bacc.Bacc("TRN2", target_bir_lowering=False, debug=False,
                   num_devices=N_CORES)

    ext = {}
    for name, shape, dt in [
        ("x_p", [P, HC * S], BF16),
        ("wq_p", [P, HC * HPC * HD], BF16),
        ("wk_p", [P, HC * HD], BF16),
        ("wv_p", [P, HC * HD], BF16),
        ("wo_p", [P, HPC * H], BF16),
        ("cos_p", [HD, S], BF16),
        ("sin_p", [HD, S], BF16),
        ("rmat", [P, P], BF16),
        ("tri", [P, P], BF16),
        ("mbias", [P, KC], F32),
        ("onescol", [P, 1], BF16),
        ("onesrow", [1, P], BF16),
    ]:
        ext[name] = nc.dram_tensor(name, shape, dt, kind="ExternalInput")
    out_ext = nc.dram_tensor("out_p", [S, H], BF16, kind="ExternalOutput")

    scale = float(1.0 / np.sqrt(HD))

    from contextlib import nullcontext
    with nc.allow_low_precision(reason="bf16 kernel; fits rel-err budget"), \
         tile.TileContext(nc) as tc:
        with tc.tile_pool(name="persist", bufs=1) as persist, \
             (tc.For_i(0, loop_n, 1,
                       hint_engines=(mybir.EngineType.PE,
                                     mybir.EngineType.Activation,
                                     mybir.EngineType.DVE,
                                     mybir.EngineType.Pool,
                                     mybir.EngineType.SP))
              if loop_n > 1 and hw_loop else nullcontext()):
          for _unroll in range(loop_n if not hw_loop else 1):
            # ---------------- persistent SBUF ----------------
            qT_all = persist.tile([P, HPC * S], BF16)   # [hd, h*S + tok]
            kT_all = persist.tile([P, S], BF16)         # [hd, tok]
            v_all = persist.tile([P, S], BF16)          # [tok%128, kc*128+hd]
            outT_all = persist.tile([P, HPC * S], BF16)  # [hd, h*S + tok]
            cos_sb = persist.tile([HD, S], BF16)
            sin_sb = persist.tile([HD, S], BF16)
            wq_sb = persist.tile([P, HC * HPC * HD], BF16)
            wk_sb = persist.tile([P, HC * HD], BF16)
            wv_sb = persist.tile([P, HC * HD], BF16)
            wo_sb = persist.tile([P, HPC * H], BF16)
            rmat_sb = persist.tile([P, P], BF16)
            tri_sb = persist.tile([P, P], BF16)
            mb_sb = persist.tile([P, KC], F32)
            ones_sb = persist.tile([P, 1], BF16)
            onesrow_sb = persist.tile([1, P], BF16)

            nc.sync.dma_start(rmat_sb[:], ext["rmat"][:])
            nc.sync.dma_start(tri_sb[:], ext["tri"][:])
            nc.sync.dma_start(mb_sb[:], ext["mbias"][:])
            nc.sync.dma_start(ones_sb[:], ext["onescol"][:])
            nc.sync.dma_start(onesrow_sb[:], ext["onesrow"][:])
            nc.sync.dma_start(cos_sb[:], ext["cos_p"][:])
            nc.sync.dma_start(sin_sb[:], ext["sin_p"][:])
            nc.sync.dma_start(wq_sb[:], ext["wq_p"][:])
            nc.sync.dma_start(wk_sb[:], ext["wk_p"][:])
            nc.sync.dma_start(wv_sb[:], ext["wv_p"][:])
            nc.sync.dma_start(wo_sb[:], ext["wo_p"][:])

            with tc.tile_pool(name="xt", bufs=2) as xtp, \
                 tc.tile_pool(name="rope", bufs=4) as rope, \
                 tc.tile_pool(name="expp", bufs=6) as expp, \
                 tc.tile_pool(name="smx", bufs=2) as smx, \
                 tc.tile_pool(name="ost", bufs=2) as ost, \
                 tc.tile_pool(name="psP", bufs=2, space="PSUM") as psP, \
                 tc.tile_pool(name="psR", bufs=1, space="PSUM") as psR, \
                 tc.tile_pool(name="psS", bufs=2, space="PSUM") as psS, \
                 tc.tile_pool(name="psO", bufs=1, space="PSUM") as psO, \
                 tc.tile_pool(name="psr", bufs=1, space="PSUM") as psr:

                def rope_store(ps_raw, dst_ap, jq):
                    """dst = raw*cos + (rmat.T @ raw)*sin  (token chunk jq)."""
                    raw = rope.tile([P, NQ], BF16, tag="raw")
                    nc.scalar.activation(raw[:], ps_raw[:], AF.Copy)
                    ps_rot = psR.tile([P, NQ], F32, tag="rot")
                    nc.tensor.matmul(ps_rot[:], rmat_sb[:], raw[:],
                                     start=True, stop=True)
                    t1 = rope.tile([P, NQ], BF16, tag="t1")
                    nc.vector.tensor_mul(
                        t1[:], raw[:], cos_sb[:, jq * NQ:(jq + 1) * NQ])
                    t2 = rope.tile([P, NQ], BF16, tag="t2")
                    nc.vector.tensor_mul(
                        t2[:], ps_rot[:], sin_sb[:, jq * NQ:(jq + 1) * NQ])
                    nc.vector.tensor_add(dst_ap, t1[:], t2[:])

                def phase_a(jq):
                    xt = xtp.tile([P, HC * NQ], BF16, tag="xt")
                    for c in range(HC):
                        nc.sync.dma_start(
                            xt[:, c * NQ:(c + 1) * NQ],
                            ext["x_p"][:, c * S + jq * NQ:
                                       c * S + jq * NQ + NQ])
                    # K projection first (B(jq) needs the diagonal K chunk)
                    ps = psP.tile([P, NQ], F32, tag="proj")
                    for c in range(HC):
                        nc.tensor.matmul(
                            ps[:], wk_sb[:, c * HD:(c + 1) * HD],
                            xt[:, c * NQ:(c + 1) * NQ],
                            start=(c == 0), stop=(c == HC - 1))
                    rope_store(ps, kT_all[:, jq * NQ:(jq + 1) * NQ], jq)
                    # V as [hd, tok] (512-moving), then DMA-transpose (XBAR)
                    # into the [tok, hd] layout PV needs - no PE transposes.
                    ps = psP.tile([P, NQ], F32, tag="proj")
                    for c in range(HC):
                        nc.tensor.matmul(
                            ps[:], wv_sb[:, c * HD:(c + 1) * HD],
                            xt[:, c * NQ:(c + 1) * NQ],
                            start=(c == 0), stop=(c == HC - 1))
                    vT = rope.tile([P, NQ], BF16, tag="vT")
                    nc.scalar.activation(vT[:], ps[:], AF.Copy)
                    for s4 in range(NQ // P):
                        kc = jq * (NQ // P) + s4
                        nc.sync.dma_start(
                            v_all[:, kc * P:(kc + 1) * P],
                            vT[:, s4 * P:(s4 + 1) * P], transpose=True)
                    # Q projections for the 4 heads
                    for h in range(HPC):
                        ps = psP.tile([P, NQ], F32, tag="proj")
                        for c in range(HC):
                            nc.tensor.matmul(
                                ps[:],
                                wq_sb[:, c * HPC * HD + h * HD:
                                      c * HPC * HD + (h + 1) * HD],
                                xt[:, c * NQ:(c + 1) * NQ],
                                start=(c == 0), stop=(c == HC - 1))
                        rope_store(
                            ps, qT_all[:, h * S + jq * NQ:h * S + (jq + 1) * NQ],
                            jq)

                def phase_b(jq):
                    for h in range(HPC):
                        nkc = (jq + 1) * (NQ // P)
                        q0 = h * S + jq * NQ
                        ps_out = psO.tile([P, NQ], F32, tag="pv")
                        ps_r = psr.tile([1, NQ], F32, tag="r")
                        pend = None

                        def drain(pkc, pqlo, pet):
                            nc.tensor.matmul(
                                ps_r[:, pqlo:NQ], ones_sb[:],
                                pet[:, pqlo:NQ],
                                start=(pkc == 0), stop=(pkc == nkc - 1))
                            nc.tensor.matmul(
                                ps_out[:, pqlo:NQ],
                                v_all[:, pkc * P:(pkc + 1) * P],
                                pet[:, pqlo:NQ],
                                start=(pkc == 0), stop=(pkc == nkc - 1))

                        for kc in range(nkc):
                            r = kc - jq * (NQ // P)   # straddle index
                            qlo = r * P if r >= 0 else 0
                            ps_sc = psS.tile([P, NQ], F32, tag="sc")
                            nc.tensor.matmul(
                                ps_sc[:, qlo:NQ],
                                kT_all[:, kc * P:(kc + 1) * P],
                                qT_all[:, q0 + qlo:q0 + NQ],
                                start=True, stop=True)
                            et = expp.tile([P, NQ], BF16, tag="exp")
                            nc.scalar.activation(
                                et[:, qlo:NQ], ps_sc[:, qlo:NQ], AF.Exp,
                                scale=scale, bias=mb_sb[:, kc:kc + 1])
                            if r >= 0:
                                nc.vector.tensor_mul(
                                    et[:, qlo:qlo + P], et[:, qlo:qlo + P],
                                    tri_sb[:])
                            if pend is not None:
                                drain(*pend)
                            pend = (kc, qlo, et)
                        drain(*pend)
                        # normalization: recip -> PE partition-broadcast ->
                        # fold into PSUM->SBUF copy
                        recip = smx.tile([1, NQ], BF16, tag="recip")
                        nc.vector.reciprocal(recip[:], ps_r[:])
                        ps_b = psR.tile([P, NQ], F32, tag="bcast")
                        nc.tensor.matmul(ps_b[:], onesrow_sb[:], recip[:],
                                         start=True, stop=True)
                        b_sb = smx.tile([P, NQ], BF16, tag="bsb")
                        nc.scalar.activation(b_sb[:], ps_b[:], AF.Copy)
                        nc.vector.tensor_mul(
                            outT_all[:, q0:q0 + NQ], ps_out[:], b_sb[:])

                def phase_c(jq):
                    for s4 in range(NQ // P):
                        tc_i = jq * (NQ // P) + s4
                        st = ost.tile([P, H], BF16, tag="st")
                        for n in range(H // NQ):
                            ps = psP.tile([P, NQ], F32, tag="proj")
                            for h in range(HPC):
                                nc.tensor.matmul(
                                    ps[:],
                                    outT_all[:, h * S + tc_i * P:
                                             h * S + (tc_i + 1) * P],
                                    wo_sb[:, h * H + n * NQ:h * H + (n + 1) * NQ],
                                    start=(h == 0), stop=(h == HPC - 1))
                            if n % 2 == 0:
                                nc.scalar.activation(
                                    st[:, n * NQ:(n + 1) * NQ], ps[:], AF.Copy)
                            else:
                                nc.vector.tensor_copy(
                                    st[:, n * NQ:(n + 1) * NQ], ps[:])
                        nc.sync.dma_start(
                            out_ext[tc_i * P:(tc_i + 1) * P, :], st[:])

                for jq in range(NJ):
                    phase_a(jq)
                    if jq > 0:
                        phase_c(jq - 1)
                    phase_b(jq)
                phase_c(NJ - 1)

    nc.compile()
    return nc


def _host_consts():
    rmat = np.zeros((P, P), dtype=np.float32)
    half = HD // 2
    for j in range(half):
        rmat[half + j, j] = -1.0      # out[j]      = -q[j+64]
        rmat[j, half + j] = 1.0       # out[j+64]   =  q[j]
    tri = np.triu(np.ones((P, P), dtype=np.float32))   # keep k_local <= q_local
    onescol = np.ones((P, 1), dtype=np.float32)
    onesrow = np.ones((1, P), dtype=np.float32)
    return rmat, tri, onescol, onesrow


def _pack_rows(a, nblk):
    """[nblk*128, F] -> [128, nblk*F] with out[p, c*F+j] = a[c*128+p, j]."""
    nb, f = a.shape
    return np.ascontiguousarray(
        a.reshape(nblk, P, f // 1).transpose(1, 0, 2).reshape(P, nblk * f))


def build_in_maps(hidden_states, cos, sin, Wq, Wk, Wv, Wo, attention_mask):
    rmat, tri, onescol, onesrow = _host_consts()
    bd = lambda a: np.ascontiguousarray(a).astype(BD)  # noqa: E731
    cos_p = bd(cos.T)
    sin_p = bd(sin.T)
    in_maps = []
    for core in range(N_CORES):
        b, g = divmod(core, G)
        xT = hidden_states[b].T.astype(np.float32)      # [H, S]
        x_p = bd(_pack_rows(xT, HC))                    # [128, HC*S]
        wq_p = bd(_pack_rows(
            Wq[:, g * HPC * HD:(g + 1) * HPC * HD].astype(np.float32), HC))
        wk_p = bd(_pack_rows(
            Wk[:, g * HD:(g + 1) * HD].astype(np.float32), HC))
        wv_p = bd(_pack_rows(
            Wv[:, g * HD:(g + 1) * HD].astype(np.float32), HC))
        wo_p = bd(_pack_rows(
            Wo[g * HPC * HD:(g + 1) * HPC * HD, :].astype(np.float32), HPC))
        mb = ((attention_mask[b].astype(np.float32) - 1.0) * 1e30)
        mb = np.ascontiguousarray(mb.reshape(KC, P).T)
        in_maps.append({
            "x_p": x_p, "wq_p": wq_p, "wk_p": wk_p, "wv_p": wv_p,
            "wo_p": wo_p, "cos_p": cos_p, "sin_p": sin_p,
            "rmat": bd(rmat), "tri": bd(tri), "mbias": mb,
            "onescol": bd(onescol), "onesrow": bd(onesrow),
        })
    return in_maps


def kernel(hidden_states, cos, sin, Wq, Wk, Wv, Wo, attention_mask):
    if "nc" not in _CACHE:
        _CACHE["nc"] = _build_program()
    nc = _CACHE["nc"]
    in_maps = build_in_maps(np.asarray(hidden_states, np.float32),
                            np.asarray(cos, np.float32),
                            np.asarray(sin, np.float32),
                            np.asarray(Wq, np.float32),
                            np.asarray(Wk, np.float32),
                            np.asarray(Wv, np.float32),
                            np.asarray(Wo, np.float32),
                            np.asarray(attention_mask, np.float32))
    res = run_bass_kernel_spmd(nc, in_maps, list(range(N_CORES)))
    out = np.empty((B, S, H), dtype=np.float32)
    for b in range(B):
        acc = np.asarray(res.results[4 * b]["out_p"]).astype(np.float32)
        for g in range(1, G):
            acc = acc + np.asarray(res.results[4 * b + g]["out_p"]).astype(
                np.float32)
        out[b] = acc
    return out


if __name__ == "__main__":
    rng = np.random.default_rng(0)
    hs = rng.standard_normal((B, S, H), dtype=np.float32)
    inv_freq = 1.0 / (10000.0 ** (np.arange(0, HD, 2, dtype=np.float32) / HD))
    t = np.arange(S, dtype=np.float32)
    freqs = np.outer(t, inv_freq)
    emb = np.concatenate([freqs, freqs], axis=-1)
    out = kernel(hs, np.cos(emb), np.sin(emb),
                 rng.standard_normal((H, NH * HD), dtype=np.float32) * 0.02,
                 rng.standard_normal((H, NKV * HD), dtype=np.float32) * 0.02,
                 rng.standard_normal((H, NKV * HD), dtype=np.float32) * 0.02,
                 rng.standard_normal((NH * HD, H), dtype=np.float32) * 0.02,
                 np.ones((B, S), dtype=np.float32))
    print("kernel ran, out shape", out.shape, "finite:", np.isfinite(out).all())


# revision 13
# speedup vs baseline: 1.1637x; 1.1637x over previous
"""Trainium2 Bass kernel for nn_Attention_59708635349115.

Decoder self-attention (GQA 16 q-heads / 4 kv-heads, RoPE, causal) over
B=2, S=2048, H=2048, distributed over 8 NeuronCores as 2 (batch) x 4
(head-group) shards.  Each core computes q/k/v projections for its
4 q-heads / 1 kv head, causal attention, and a partial o-projection
against its 512-row slice of Wo; the host sums the 4 partials per batch
(no on-device collectives).

v2 design vs the fp32r baseline:
  - bf16 operands everywhere (PSUM accumulation stays fp32); the host
    pre-packs every input into its exact SBUF layout so each tensor is
    one (or few) large contiguous DMA.
  - V is projected directly into [token, head_dim] layout by making the
    x chunk the stationary matmul operand - no PE transposes at all.
  - softmax normalization: 1/r is broadcast across partitions with a
    1-partition PE matmul (ones_row x recip_row) instead of the slow
    gpsimd partition_broadcast, then folded into the PSUM->SBUF copy of
    the PV output on DVE.
  - elementwise work split across DVE (RoPE muls, tri mask, normalize)
    and Pool/gpsimd (RoPE sin-mul, o_proj PSUM drain) so ACT only runs
    exp + a few copies (all in the same activation table - no reloads).
  - phases are software-pipelined per 512-token chunk jq:
    A(jq) projections -> C(jq-1) o-proj -> B(jq) attention, keeping PE
    fed across cross-engine dependency chains.
"""

import os
import sys

for _p in ("/opt/trn_rl_repo", "/root/.axon_site/_ro/trn_rl_repo"):
    if os.path.isdir(_p) and _p not in sys.path:
        sys.path.insert(0, _p)

import numpy as np
import ml_dtypes

import concourse.bass as bass
import concourse.mybir as mybir
import concourse.tile as tile
from concourse import bacc
from concourse.bass_utils import run_bass_kernel_spmd

B, S, H = 2, 2048, 2048
NH, NKV = 16, 4
HD = H // NH            # 128
G = 4                   # head-group shards (tensor parallel)
HPC = NH // G           # 4 q heads per core
N_CORES = 8
P = 128                 # partition dim
NQ = 512                # q-chunk (matmul moving dim)
NJ = S // NQ            # 4 q-chunks
KC = S // P             # 16 key/token 128-chunks
HC = H // P             # 16 hidden 128-chunks

F32 = mybir.dt.float32
F32R = mybir.dt.float32r
BF16 = mybir.dt.bfloat16
AF = mybir.ActivationFunctionType
BD = ml_dtypes.bfloat16

_CACHE = {}


def _build_program(loop_n=1, hw_loop=True):
    nc = bacc.Bacc("TRN2", target_bir_lowering=False, debug=False,
                   num_devices=N_CORES)

    ext = {}
    for name, shape, dt in [
        ("x_p", [P, HC * S], BF16),
        ("wq_p", [P, HC * HPC * HD], BF16),
        ("wk_p", [P, HC * HD], BF16),
        ("wv_p", [P, HC * HD], BF16),
        ("wo_p", [P, HPC * H], F32),
        ("cos_p", [HD, S], BF16),
        ("sin_p", [HD, S], BF16),
        ("rmat", [P, P], BF16),
        ("tri", [P, P], BF16),
        ("mbias", [P, KC], F32),
        ("onescol", [P, 1], BF16),
        ("onesrow", [1, P], BF16),
    ]:
        ext[name] = nc.dram_tensor(name, shape, dt, kind="ExternalInput")
    out_ext = nc.dram_tensor("out_p", [S, H], BF16, kind="ExternalOutput")

    scale = float(1.0 / np.sqrt(HD))

    from contextlib import nullcontext
    with nc.allow_low_precision(reason="bf16 kernel; fits rel-err budget"), \
         tile.TileContext(nc) as tc:
        with tc.tile_pool(name="persist", bufs=1) as persist, \
             (tc.For_i(0, loop_n, 1,
                       hint_engines=(mybir.EngineType.PE,
                                     mybir.EngineType.Activation,
                                     mybir.EngineType.DVE,
                                     mybir.EngineType.Pool,
                                     mybir.EngineType.SP))
              if loop_n > 1 and hw_loop else nullcontext()):
          for _unroll in range(loop_n if not hw_loop else 1):
            # ---------------- persistent SBUF ----------------
            kT_all = persist.tile([P, S], F32R)         # [hd, tok]
            v_all = persist.tile([P, S], BF16)          # [tok%128, kc*128+hd]
            cos_sb = persist.tile([HD, S], BF16)
            sin_sb = persist.tile([HD, S], BF16)
            wq_sb = persist.tile([P, HC * HPC * HD], BF16)
            wk_sb = persist.tile([P, HC * HD], BF16)
            wv_sb = persist.tile([P, HC * HD], BF16)
            wo_sb = persist.tile([P, HPC * H], F32R)
            rmat_sb = persist.tile([P, P], BF16)
            tri_sb = persist.tile([P, P], BF16)
            mb_sb = persist.tile([P, KC], F32)
            ones_sb = persist.tile([P, 1], BF16)
            onesrow_sb = persist.tile([1, P], BF16)

            nc.sync.dma_start(rmat_sb[:], ext["rmat"][:])
            nc.sync.dma_start(tri_sb[:], ext["tri"][:])
            nc.sync.dma_start(mb_sb[:], ext["mbias"][:])
            nc.sync.dma_start(ones_sb[:], ext["onescol"][:])
            nc.sync.dma_start(onesrow_sb[:], ext["onesrow"][:])
            nc.sync.dma_start(cos_sb[:], ext["cos_p"][:])
            nc.sync.dma_start(sin_sb[:], ext["sin_p"][:])
            nc.sync.dma_start(wq_sb[:], ext["wq_p"][:])
            nc.sync.dma_start(wk_sb[:], ext["wk_p"][:])
            nc.sync.dma_start(wv_sb[:], ext["wv_p"][:])
            nc.sync.dma_start(wo_sb[:], ext["wo_p"][:].bitcast(F32R))

            with tc.tile_pool(name="xt", bufs=2) as xtp, \
                 tc.tile_pool(name="qt", bufs=2) as qtp, \
                 tc.tile_pool(name="ot", bufs=2) as otp, \
                 tc.tile_pool(name="rope", bufs=4) as rope, \
                 tc.tile_pool(name="expp", bufs=6) as expp, \
                 tc.tile_pool(name="smx", bufs=2) as smx, \
                 tc.tile_pool(name="ost", bufs=2) as ost, \
                 tc.tile_pool(name="psP", bufs=2, space="PSUM") as psP, \
                 tc.tile_pool(name="psR", bufs=1, space="PSUM") as psR, \
                 tc.tile_pool(name="psS", bufs=2, space="PSUM") as psS, \
                 tc.tile_pool(name="psO", bufs=2, space="PSUM") as psO, \
                 tc.tile_pool(name="psr", bufs=1, space="PSUM") as psr:

                def rope_store(ps_raw, dst_ap, jq):
                    """dst = raw*cos + (rmat.T @ raw)*sin  (token chunk jq)."""
                    raw = rope.tile([P, NQ], BF16, tag="raw")
                    nc.scalar.activation(raw[:], ps_raw[:], AF.Copy)
                    ps_rot = psR.tile([P, NQ], F32, tag="rot")
                    nc.tensor.matmul(ps_rot[:], rmat_sb[:], raw[:],
                                     start=True, stop=True)
                    t1 = rope.tile([P, NQ], F32, tag="t1")
                    nc.vector.tensor_mul(
                        t1[:], raw[:], cos_sb[:, jq * NQ:(jq + 1) * NQ])
                    t2 = rope.tile([P, NQ], F32, tag="t2")
                    nc.vector.tensor_mul(
                        t2[:], ps_rot[:], sin_sb[:, jq * NQ:(jq + 1) * NQ])
                    nc.vector.tensor_add(dst_ap, t1[:], t2[:])

                def phase_a(jq):
                    qt = qtp.tile([P, HPC * NQ], F32R, tag="qt")
                    xt = xtp.tile([P, HC * NQ], BF16, tag="xt")
                    for c in range(HC):
                        nc.sync.dma_start(
                            xt[:, c * NQ:(c + 1) * NQ],
                            ext["x_p"][:, c * S + jq * NQ:
                                       c * S + jq * NQ + NQ])
                    # K projection first (B(jq) needs the diagonal K chunk)
                    ps = psP.tile([P, NQ], F32, tag="proj")
                    for c in range(HC):
                        nc.tensor.matmul(
                            ps[:], wk_sb[:, c * HD:(c + 1) * HD],
                            xt[:, c * NQ:(c + 1) * NQ],
                            start=(c == 0), stop=(c == HC - 1))
                    rope_store(ps, kT_all[:, jq * NQ:(jq + 1) * NQ], jq)
                    # V as [hd, tok] (512-moving), then DMA-transpose (XBAR)
                    # into the [tok, hd] layout PV needs - no PE transposes.
                    ps = psP.tile([P, NQ], F32, tag="proj")
                    for c in range(HC):
                        nc.tensor.matmul(
                            ps[:], wv_sb[:, c * HD:(c + 1) * HD],
                            xt[:, c * NQ:(c + 1) * NQ],
                            start=(c == 0), stop=(c == HC - 1))
                    vT = rope.tile([P, NQ], BF16, tag="vT")
                    nc.scalar.activation(vT[:], ps[:], AF.Copy)
                    for s4 in range(NQ // P):
                        kc = jq * (NQ // P) + s4
                        nc.sync.dma_start(
                            v_all[:, kc * P:(kc + 1) * P],
                            vT[:, s4 * P:(s4 + 1) * P], transpose=True)
                    # Q projections for the 4 heads
                    for h in range(HPC):
                        ps = psP.tile([P, NQ], F32, tag="proj")
                        for c in range(HC):
                            nc.tensor.matmul(
                                ps[:],
                                wq_sb[:, c * HPC * HD + h * HD:
                                      c * HPC * HD + (h + 1) * HD],
                                xt[:, c * NQ:(c + 1) * NQ],
                                start=(c == 0), stop=(c == HC - 1))
                        rope_store(ps, qt[:, h * NQ:(h + 1) * NQ], jq)
                    return qt

                def phase_b(jq, qt):
                    ot = otp.tile([P, HPC * NQ], F32R, tag="ot")
                    for h in range(HPC):
                        nkc = (jq + 1) * (NQ // P)
                        q0 = h * NQ
                        ps_out = psO.tile([P, NQ], F32, tag="pv")
                        ps_r = psr.tile([1, NQ], F32, tag="r")
                        pend = None

                        def drain(pkc, pqlo, pet):
                            nc.tensor.matmul(
                                ps_r[:, pqlo:NQ], ones_sb[:],
                                pet[:, pqlo:NQ],
                                start=(pkc == 0), stop=(pkc == nkc - 1))
                            nc.tensor.matmul(
                                ps_out[:, pqlo:NQ],
                                v_all[:, pkc * P:(pkc + 1) * P],
                                pet[:, pqlo:NQ],
                                start=(pkc == 0), stop=(pkc == nkc - 1))

                        for kc in range(nkc):
                            r = kc - jq * (NQ // P)   # straddle index
                            qlo = r * P if r >= 0 else 0
                            ps_sc = psS.tile([P, NQ], F32, tag="sc")
                            nc.tensor.matmul(
                                ps_sc[:, qlo:NQ],
                                kT_all[:, kc * P:(kc + 1) * P],
                                qt[:, q0 + qlo:q0 + NQ],
                                start=True, stop=True)
                            et = expp.tile([P, NQ], BF16, tag="exp")
                            nc.scalar.activation(
                                et[:, qlo:NQ], ps_sc[:, qlo:NQ], AF.Exp,
                                scale=scale, bias=mb_sb[:, kc:kc + 1])
                            if r >= 0:
                                nc.vector.tensor_mul(
                                    et[:, qlo:qlo + P], et[:, qlo:qlo + P],
                                    tri_sb[:])
                            if pend is not None:
                                drain(*pend)
                            pend = (kc, qlo, et)
                        drain(*pend)
                        # normalization: recip -> PE partition-broadcast ->
                        # fold into PSUM->SBUF copy
                        recip = smx.tile([1, NQ], BF16, tag="recip")
                        nc.vector.reciprocal(recip[:], ps_r[:])
                        ps_b = psR.tile([P, NQ], F32, tag="rot")
                        nc.tensor.matmul(ps_b[:], onesrow_sb[:], recip[:],
                                         start=True, stop=True)
                        b_sb = smx.tile([P, NQ], BF16, tag="bsb")
                        nc.scalar.activation(b_sb[:], ps_b[:], AF.Copy)
                        nc.vector.tensor_mul(
                            ot[:, q0:q0 + NQ], ps_out[:], b_sb[:])
                    return ot

                def phase_c(jq, ot):
                    for s4 in range(NQ // P):
                        tc_i = jq * (NQ // P) + s4
                        st = ost.tile([P, H], BF16, tag="st")
                        for n in range(H // NQ):
                            ps = psP.tile([P, NQ], F32, tag="proj")
                            for h in range(HPC):
                                nc.tensor.matmul(
                                    ps[:],
                                    ot[:, h * NQ + s4 * P:
                                       h * NQ + (s4 + 1) * P],
                                    wo_sb[:, h * H + n * NQ:h * H + (n + 1) * NQ],
                                    start=(h == 0), stop=(h == HPC - 1))
                            if n % 2 == 0:
                                nc.scalar.activation(
                                    st[:, n * NQ:(n + 1) * NQ], ps[:], AF.Copy)
                            else:
                                nc.vector.tensor_copy(
                                    st[:, n * NQ:(n + 1) * NQ], ps[:])
                        nc.sync.dma_start(
                            out_ext[tc_i * P:(tc_i + 1) * P, :], st[:])

                ablate = os.environ.get("BASS_ABLATE", "")
                prev_ot = None
                for jq in range(NJ):
                    qt = phase_a(jq)
                    if ablate == "A":
                        continue
                    if prev_ot is not None and ablate != "noC":
                        phase_c(jq - 1, prev_ot)
                    if ablate == "noB":
                        prev_ot = otp.tile([P, HPC * NQ], F32R, tag="ot")
                        nc.vector.tensor_copy(prev_ot[:, 0:NQ], qt[:, 0:NQ])
                    else:
                        prev_ot = phase_b(jq, qt)
                if ablate != "A" and ablate != "noC":
                    phase_c(NJ - 1, prev_ot)

    nc.compile()
    return nc


def _host_consts():
    rmat = np.zeros((P, P), dtype=np.float32)
    half = HD // 2
    for j in range(half):
        rmat[half + j, j] = -1.0      # out[j]      = -q[j+64]
        rmat[j, half + j] = 1.0       # out[j+64]   =  q[j]
    tri = np.triu(np.ones((P, P), dtype=np.float32))   # keep k_local <= q_local
    onescol = np.ones((P, 1), dtype=np.float32)
    onesrow = np.ones((1, P), dtype=np.float32)
    return rmat, tri, onescol, onesrow


def _pack_rows(a, nblk):
    """[nblk*128, F] -> [128, nblk*F] with out[p, c*F+j] = a[c*128+p, j]."""
    nb, f = a.shape
    return np.ascontiguousarray(
        a.reshape(nblk, P, f // 1).transpose(1, 0, 2).reshape(P, nblk * f))


def build_in_maps(hidden_states, cos, sin, Wq, Wk, Wv, Wo, attention_mask):
    rmat, tri, onescol, onesrow = _host_consts()
    bd = lambda a: np.ascontiguousarray(a).astype(BD)  # noqa: E731
    cos_p = bd(cos.T)
    sin_p = bd(sin.T)
    in_maps = []
    for core in range(N_CORES):
        b, g = divmod(core, G)
        xT = hidden_states[b].T.astype(np.float32)      # [H, S]
        x_p = bd(_pack_rows(xT, HC))                    # [128, HC*S]
        wq_p = bd(_pack_rows(
            Wq[:, g * HPC * HD:(g + 1) * HPC * HD].astype(np.float32), HC))
        wk_p = bd(_pack_rows(
            Wk[:, g * HD:(g + 1) * HD].astype(np.float32), HC))
        wv_p = bd(_pack_rows(
            Wv[:, g * HD:(g + 1) * HD].astype(np.float32), HC))
        wo_p = np.ascontiguousarray(_pack_rows(
            Wo[g * HPC * HD:(g + 1) * HPC * HD, :].astype(np.float32), HPC))
        mb = ((attention_mask[b].astype(np.float32) - 1.0) * 1e30)
        mb = np.ascontiguousarray(mb.reshape(KC, P).T)
        in_maps.append({
            "x_p": x_p, "wq_p": wq_p, "wk_p": wk_p, "wv_p": wv_p,
            "wo_p": wo_p, "cos_p": cos_p, "sin_p": sin_p,
            "rmat": bd(rmat), "tri": bd(tri), "mbias": mb,
            "onescol": bd(onescol), "onesrow": bd(onesrow),
        })
    return in_maps


def kernel(hidden_states, cos, sin, Wq, Wk, Wv, Wo, attention_mask):
    if "nc" not in _CACHE:
        _CACHE["nc"] = _build_program()
    nc = _CACHE["nc"]
    in_maps = build_in_maps(np.asarray(hidden_states, np.float32),
                            np.asarray(cos, np.float32),
                            np.asarray(sin, np.float32),
                            np.asarray(Wq, np.float32),
                            np.asarray(Wk, np.float32),
                            np.asarray(Wv, np.float32),
                            np.asarray(Wo, np.float32),
                            np.asarray(attention_mask, np.float32))
    res = run_bass_kernel_spmd(nc, in_maps, list(range(N_CORES)))
    out = np.empty((B, S, H), dtype=np.float32)
    for b in range(B):
        acc = np.asarray(res.results[4 * b]["out_p"]).astype(np.float32)
        for g in range(1, G):
            acc = acc + np.asarray(res.results[4 * b + g]["out_p"]).astype(
                np.float32)
        out[b] = acc
    return out


if __name__ == "__main__":
    rng = np.random.default_rng(0)
    hs = rng.standard_normal((B, S, H), dtype=np.float32)
    inv_freq = 1.0 / (10000.0 ** (np.arange(0, HD, 2, dtype=np.float32) / HD))
    t = np.arange(S, dtype=np.float32)
    freqs = np.outer(t, inv_freq)
    emb = np.concatenate([freqs, freqs], axis=-1)
    out = kernel(hs, np.cos(emb), np.sin(emb),
                 rng.standard_normal((H, NH * HD), dtype=np.float32) * 0.02,
                 rng.standard_normal((H, NKV * HD), dtype=np.float32) * 0.02,
                 rng.standard_normal((H, NKV * HD), dtype=np.float32) * 0.02,
                 rng.standard_normal((NH * HD, H), dtype=np.float32) * 0.02,
                 np.ones((B, S), dtype=np.float32))
    print("kernel ran, out shape", out.shape, "finite:", np.isfinite(out).all())
